# revision 45
# baseline (speedup 1.0000x reference)
"""GCN message-passing kernel for Trainium2 (8 NeuronCores, SPMD).

Math (matches the reference):
    gf   = RF @ W_g                          (2048, 3)   gate features
    H_k  = RF @ W_k                          (2048, 4096) per edge type k in {0,1,2}
    gate(e) = sigmoid(gf[src_e, k_e] + b_glab[p_e])
    upd[t]  = sum_{e->t} gate(e) * (H_{k_e}[src_e] + b_lab[p_e])
    out  = relu(upd)

Each image's graph is self-contained (32 regions/image, 20 edges/image),
so with 4 images per 128-row block only the *unique source regions* of a
block's edges (~60, max observed 67) need H_k rows for edge types
k=0,1.  Two blocks' unique sources are packed into one 128-row
"supertile" (8 supertiles per k instead of 16 full blocks), cutting the
dominant RF@W GEMM from 48 to 32 tile passes:
    C_k(pair)  = packed_sources(pair) @ W_k          (128, 512/core)
    out_b      = sum_k T''_k(b) @ C_k + G(b) @ b_lab + diag(g2) H_2(b)
where T''_k (128x128, zero-padded) carries gate values at
(target, packed_source_row) and is built ON DEVICE from gf with one-hot
constant matrices (host only prepares 0/1 index matrices and the packed
gather of RF rows; all data-dependent FLOPs run on Trainium).
The self-loop type k=2 touches every region, so H_2 stays a full
per-block GEMM, interleaved with the gf matmuls that share its lhsT.

Sharding: the output D dim (4096) is split 8 ways -> each core computes
all 2048 rows x its 512 columns, holding a (4096 x 3*512) slice of
W_conv.  No collectives needed; host concatenates the column slices.
"""

import numpy as np
import ml_dtypes

# problem constants (hardcoded per contract)
N_IMG = 64
REG = 32
RPI = 32
NUM_REL = 20
D = 4096
NPRED = 81
N = N_IMG * REG          # 2048
NCORES = 8
CW = D // NCORES         # 512 output cols per core
NBLK = N // 128          # 16 row blocks
NPAIR = NBLK // 2        # 8 supertile pairs
IPB = 128 // REG         # 4 images per block
EPB = IPB * NUM_REL      # 80 edges per block per edge type

BF = ml_dtypes.bfloat16

_prog_cache = {}


def _build_program():
    import concourse.bass as bass
    import concourse.tile as tile
    from concourse import bacc, mybir

    bf16 = mybir.dt.bfloat16
    f32 = mybir.dt.float32
    AF = mybir.ActivationFunctionType
    ALU = mybir.AluOpType

    nc = bacc.Bacc("TRN2", target_bir_lowering=False, debug=False,
                   num_devices=NCORES)

    QW = 8 * 128  # quarter-block tile width (8 d-tiles)
    rft = nc.dram_tensor("rft", [NBLK, 128, 32 * 128], bf16, kind="ExternalInput").ap()
    gat = nc.dram_tensor("gat", [NPAIR * 2, 128, 32 * 128], bf16, kind="ExternalInput").ap()
    w = nc.dram_tensor("w", [128, 3 * 32 * CW], bf16, kind="ExternalInput").ap()
    wg = nc.dram_tensor("wg", [128, 32 * 3], bf16, kind="ExternalInput").ap()
    blab = nc.dram_tensor("blab", [NPRED, CW], bf16, kind="ExternalInput").ap()
    bgb = nc.dram_tensor("bgb", [128, NPRED], bf16, kind="ExternalInput").ap()
    srct = nc.dram_tensor("srct", [128, NBLK * 2 * EPB], bf16, kind="ExternalInput").ap()
    uco = nc.dram_tensor("uco", [EPB, NBLK * 2 * 128], bf16, kind="ExternalInput").ap()
    tgto = nc.dram_tensor("tgto", [EPB, NBLK * 2 * 128], bf16, kind="ExternalInput").ap()
    p1h = nc.dram_tensor("p1h", [EPB, NBLK * NPRED], bf16, kind="ExternalInput").ap()
    p1hs = nc.dram_tensor("p1hs", [128, NPRED], bf16, kind="ExternalInput").ap()
    ident = nc.dram_tensor("ident", [128, 128], bf16, kind="ExternalInput").ap()
    # bf16 output halves the write-drain; host upcasts to f32 (the extra
    # ~0.2% rounding is far inside the accuracy gate)
    out = nc.dram_tensor("out", [NBLK, 128, CW], bf16, kind="ExternalOutput").ap()

    with tile.TileContext(nc) as tc:
        with (
            tc.tile_pool(name="consts", bufs=1) as cpool,
            tc.tile_pool(name="rftq", bufs=12) as rpool,
            tc.tile_pool(name="gatq", bufs=8) as gpool,
            tc.tile_pool(name="csb", bufs=3) as cspool,
            tc.tile_pool(name="blk", bufs=NBLK) as bpool,
            tc.tile_pool(name="bld", bufs=NBLK) as dpool,
            tc.tile_pool(name="small", bufs=2) as spool,
            tc.tile_pool(name="osb", bufs=2) as opool,
            tc.tile_pool(name="pc", bufs=2, space="PSUM") as pcp,
            tc.tile_pool(name="pgf", bufs=1, space="PSUM") as pgfp,
            tc.tile_pool(name="prg", bufs=1, space="PSUM") as prgp,
            tc.tile_pool(name="pgt", bufs=1, space="PSUM") as pgtp,
            tc.tile_pool(name="pmt", bufs=1, space="PSUM") as pmtp,
            tc.tile_pool(name="pout", bufs=2, space="PSUM") as poutp,
        ):
            # --- w chunks: 4 d-tiles each, per k; fine-grained for early start
            WCH = 4 * CW
            w_ch = [[cpool.tile([128, WCH], bf16, tag=f"w{k}c{g}",
                                name=f"w{k}c{g}") for g in range(8)]
                    for k in range(3)]

            def _load_wk(k, eng):
                for g in range(8):
                    eng.dma_start(
                        out=w_ch[k][g][:],
                        in_=w[:, (k * 32 + 4 * g) * CW:(k * 32 + 4 * g + 4) * CW])

            rft_tiles, gat_tiles = {}, {}

            def _load_rft(b, eng=None):
                eng = eng or nc.sync
                qs = []
                for q in range(4):
                    t = rpool.tile([128, QW], bf16, tag="rftq",
                                   name=f"rft{b}_{q}")
                    eng.dma_start(out=t[:],
                                  in_=rft[b, :, q * QW:(q + 1) * QW])
                    qs.append(t)
                rft_tiles[b] = qs

            def _load_gat(i, k):
                qs = []
                for q in range(4):
                    t = gpool.tile([128, QW], bf16, tag="gatq",
                                   name=f"gat{i}_{k}_{q}")
                    nc.sync.dma_start(out=t[:],
                                      in_=gat[i * 2 + k, :, q * QW:(q + 1) * QW])
                    qs.append(t)
                gat_tiles[(i, k)] = qs

            # --- input DMAs on two HW queues:
            #  sync queue:   w2 + the rft/gat bulk streams in consumption
            #                order (pool-ring throttled)
            #  scalar queue: gate-build consts first (cheap, needed by the
            #                builds interleaved into phase A), then W0/W1;
            #                immune to the bulk stream's head-of-line
            #                throttling, arrives during the H2 phase ---
            wg_sb = cpool.tile([128, 32 * 3], bf16, tag="wg")
            nc.scalar.dma_start(out=wg_sb[:], in_=wg[:])
            bgb_sb = cpool.tile([128, NPRED], bf16, tag="bgb")
            nc.scalar.dma_start(out=bgb_sb[:], in_=bgb[:])
            p1hs_sb = cpool.tile([128, NPRED], bf16, tag="p1hs")
            nc.scalar.dma_start(out=p1hs_sb[:], in_=p1hs[:])
            ident_sb = cpool.tile([128, 128], bf16, tag="ident")
            nc.scalar.dma_start(out=ident_sb[:], in_=ident[:])
            blab_sb = cpool.tile([NPRED, CW], bf16, tag="blab")
            nc.scalar.dma_start(out=blab_sb[:], in_=blab[:])
            srct_sb = cpool.tile([128, NBLK * 2 * EPB], bf16, tag="srct")
            nc.scalar.dma_start(out=srct_sb[:], in_=srct[:])
            uco_sb = cpool.tile([EPB, NBLK * 2 * 128], bf16, tag="uco")
            nc.scalar.dma_start(out=uco_sb[:], in_=uco[:])
            tgto_sb = cpool.tile([EPB, NBLK * 2 * 128], bf16, tag="tgto")
            nc.scalar.dma_start(out=tgto_sb[:], in_=tgto[:])
            p1h_sb = cpool.tile([EPB, NBLK * NPRED], bf16, tag="p1h")
            nc.scalar.dma_start(out=p1h_sb[:], in_=p1h[:])
            # interleave rft0 with w2 so block 0's two sub-accumulations
            # gate on ~2.6MB / ~5.2MB instead of all of w2+rft0
            rft0_q = []
            def _load_rft0_q(q):
                t = rpool.tile([128, QW], bf16, tag="rftq", name=f"rft0_{q}")
                nc.sync.dma_start(out=t[:], in_=rft[0, :, q * QW:(q + 1) * QW])
                rft0_q.append(t)
            _load_rft0_q(0)
            _load_rft0_q(1)
            for g in range(4):
                nc.sync.dma_start(
                    out=w_ch[2][g][:],
                    in_=w[:, (2 * 32 + 4 * g) * CW:(2 * 32 + 4 * g + 4) * CW])
            _load_rft0_q(2)
            _load_rft0_q(3)
            for g in range(4, 8):
                nc.sync.dma_start(
                    out=w_ch[2][g][:],
                    in_=w[:, (2 * 32 + 4 * g) * CW:(2 * 32 + 4 * g + 4) * CW])
            rft_tiles[0] = rft0_q
            for b in range(1, NBLK):
                _load_rft(b)
            # W0/W1 ride the sync queue behind the rft stream: the rft ring
            # throttling delays them past phase A's critical window, still
            # well before the first c_pass needs them.
            _load_wk(0, nc.sync)
            _load_wk(1, nc.sync)
            for i in range(NPAIR):
                _load_gat(i, 0)
                _load_gat(i, 1)

            gf_tiles, g2_tiles, h2s_tiles = {}, {}, {}
            c_sb, tT_tiles, gt_tiles = {}, {}, {}

            def rft_lhsT(b, d):
                return rft_tiles[b][d // 8][:, (d % 8) * 128:(d % 8 + 1) * 128]

            def gat_lhsT(i, k, d):
                return gat_tiles[(i, k)][d // 8][:, (d % 8) * 128:(d % 8 + 1) * 128]

            from concourse.tile_rust import add_dep_helper

            # Global PE ordering chain: the static tile scheduler otherwise
            # reorders PE instructions against the DMA arrival plan, creating
            # avoidable stalls.  sync=False deps are compile-time ordering
            # only (no hardware semaphores).
            pe_prev = [None]

            def mm(*args, chain=True, **kw):
                nc.tensor.matmul(*args, **kw)
                inst = nc.main_func.blocks[-1].instructions[-1]
                assert inst.opcode == "Matmult"
                if chain:
                    if pe_prev[0] is not None:
                        add_dep_helper(inst, pe_prev[0], sync=False,
                                       reason="pe-order")
                    pe_prev[0] = inst
                return inst

            def h2gf_pass(b, nsub=1):
                """H_2(b) = RF_b @ W_2 with gf(b) interleaved.  The gf
                matmul reuses the H matmul's stationary operand (same rft
                tile) via ldweights=False; the ordering chain keeps each gf
                adjacent to its H partner so the weights are still loaded
                when it executes.  The self-loop gate is folded into the
                PSUM->SBUF copy: h2s = diag(g2) @ H_2.

                nsub>1 splits the accumulation into sub PSUM groups so the
                pass's DMA gate is per-piece (only used for block 0, where
                the hoisted whole-pass wait would stall startup).  Subs 2/3
                borrow the pout pool's banks, idle during phase A."""
                span = 32 // nsub
                phs = []
                pgf_t = pgfp.tile([128, 12], f32, tag="pgf", name=f"pgf{b}")
                for s in range(nsub):
                    pool, ptag = (pcp, "pc") if s < 2 else (poutp, "pout")
                    ph_t = pool.tile([128, CW], f32, tag=ptag,
                                     name=f"ph{b}_{s}")
                    pgf_ap = pgf_t[:, s * 3:(s + 1) * 3]
                    for dd in range(span):
                        d = s * span + dd
                        lhsT = rft_lhsT(b, d)
                        mm(ph_t[:], lhsT,
                           w_ch[2][d // 4][:, (d % 4) * CW:(d % 4 + 1) * CW],
                           start=(dd == 0), stop=(dd == span - 1))
                        gf_inst = mm(pgf_ap, lhsT,
                                     wg_sb[:, d * 3:(d + 1) * 3],
                                     start=(dd == 0), stop=(dd == span - 1))
                        gf_inst.ldweights = False
                    phs.append(ph_t)
                gf_sb = bpool.tile([128, 3], f32, tag="gf", name=f"gf{b}")
                nc.vector.tensor_copy(out=gf_sb[:], in_=pgf_t[:, 0:3])
                for s in range(1, nsub):
                    nc.vector.tensor_add(gf_sb[:], gf_sb[:],
                                         pgf_t[:, s * 3:(s + 1) * 3])
                gf_tiles[b] = gf_sb
                g2 = bpool.tile([128, 1], f32, tag="g2", name=f"g2_{b}")
                nc.scalar.activation(g2[:], bgb_sb[:, 0:1], AF.Sigmoid,
                                     bias=gf_sb[:, 2:3])
                g2_tiles[b] = g2
                h2s = bpool.tile([128, CW], bf16, tag="h2s", name=f"h2s{b}")
                nc.vector.tensor_scalar_mul(h2s[:], phs[0][:], g2[:])
                for s in range(1, nsub):
                    tmp = spool.tile([128, CW], bf16, tag="h2tmp",
                                     name=f"h2tmp{b}_{s}")
                    nc.vector.tensor_scalar_mul(tmp[:], phs[s][:], g2[:])
                    nc.vector.tensor_add(h2s[:], h2s[:], tmp[:])
                h2s_tiles[b] = h2s
                del rft_tiles[b]

            def c_pass(i, k):
                """C_k(i) = packed_sources(i, k) @ W_k  (supertile GEMM)."""
                pc_t = pcp.tile([128, CW], f32, tag="pc", name=f"pc{i}_{k}")
                for d in range(32):
                    mm(pc_t[:], gat_lhsT(i, k, d),
                       w_ch[k][d // 4][:, (d % 4) * CW:(d % 4 + 1) * CW],
                       start=(d == 0), stop=(d == 31))
                ck = cspool.tile([128, CW], bf16, tag=f"c{k}", name=f"c{i}_{k}")
                nc.vector.tensor_copy(out=ck[:], in_=pc_t[:])
                c_sb[(i, k)] = ck
                del gat_tiles[(i, k)]

            def build(b):
                """Gates -> packed-row scatter matrices T''_k and G^T."""
                gf_sb = gf_tiles[b]
                pgt_t = pgtp.tile([NPRED, 128], f32, tag="pgt", name=f"pgt{b}")
                for k in range(2):
                    sg = spool.tile([128, NPRED], bf16, tag=f"sig{k}",
                                    name=f"sig{b}_{k}")
                    nc.scalar.activation(sg[:], bgb_sb[:], AF.Sigmoid,
                                         bias=gf_sb[:, k:k + 1])
                    prg_t = prgp.tile([EPB, NPRED], f32, tag="prg",
                                      name=f"prg{b}_{k}")
                    mm(prg_t[:],
                       srct_sb[:, (b * 2 + k) * EPB:(b * 2 + k + 1) * EPB],
                       sg[:], start=True, stop=True, chain=False)
                    pg = spool.tile([EPB, NPRED], bf16, tag="pg",
                                    name=f"pg{b}_{k}")
                    nc.vector.tensor_mul(
                        pg[:], prg_t[:],
                        p1h_sb[:, b * NPRED:(b + 1) * NPRED])
                    mm(pgt_t[:], pg[:],
                       tgto_sb[:, (b * 2 + k) * 128:(b * 2 + k + 1) * 128],
                       start=(k == 0), stop=False, chain=False)
                    gcol = spool.tile([EPB, 1], f32, tag="gcol",
                                      name=f"gcol{b}_{k}")
                    nc.vector.tensor_reduce(gcol[:], pg[:],
                                            axis=mybir.AxisListType.X,
                                            op=ALU.add)
                    ucog = spool.tile([EPB, 128], bf16, tag="ucog",
                                      name=f"ucog{b}_{k}")
                    nc.vector.tensor_scalar_mul(
                        ucog[:],
                        uco_sb[:, (b * 2 + k) * 128:(b * 2 + k + 1) * 128],
                        gcol[:])
                    pmt_t = pmtp.tile([128, 128], f32, tag="pmt",
                                      name=f"pmt{b}_{k}")
                    mm(pmt_t[:], ucog[:],
                       tgto_sb[:, (b * 2 + k) * 128:(b * 2 + k + 1) * 128],
                       start=True, stop=True, chain=False)
                    tT = dpool.tile([128, 128], bf16, tag=f"t{k}",
                                    name=f"tT{b}_{k}")
                    nc.vector.tensor_copy(out=tT[:], in_=pmt_t[:])
                    tT_tiles[(b, k)] = tT
                # self-loop: G row 0 += g2
                pg2 = spool.tile([128, NPRED], bf16, tag="pg2", name=f"pg2_{b}")
                nc.vector.tensor_scalar_mul(pg2[:], p1hs_sb[:], g2_tiles[b][:])
                mm(pgt_t[:], pg2[:], ident_sb[:], start=False, stop=True,
                   chain=False)
                gt_sb = dpool.tile([NPRED, 128], bf16, tag="gt", name=f"gt{b}")
                nc.vector.tensor_copy(out=gt_sb[:], in_=pgt_t[:])
                gt_tiles[b] = gt_sb

            def asm(b, i):
                pout_t = poutp.tile([128, CW], f32, tag="pout", name=f"po{b}")
                for k in range(2):
                    mm(pout_t[:], tT_tiles[(b, k)][:], c_sb[(i, k)][:],
                       start=(k == 0), stop=False, chain=False)
                mm(pout_t[:], gt_tiles[b][:], blab_sb[:],
                   start=False, stop=True, chain=False)
                nc.vector.tensor_add(pout_t[:], pout_t[:], h2s_tiles[b][:])
                out_sb = opool.tile([128, CW], bf16, tag="out", name=f"ob{b}")
                nc.scalar.activation(out_sb[:], pout_t[:], AF.Relu)
                nc.scalar.dma_start(out=out[b], in_=out_sb[:])
                del tT_tiles[(b, 0)], tT_tiles[(b, 1)], gt_tiles[b]
                del gf_tiles[b], g2_tiles[b], h2s_tiles[b]
                if b % 2 == 1:
                    del c_sb[(i, 0)], c_sb[(i, 1)]

            # --- phase A: all H2+gf passes (only need w2 + the rft stream;
            # cheapest PE-work-per-DMA-byte, hides the W0/W1/gat/const
            # DMAs).  Gate builds interleave here: they only need gf +
            # consts, and their outputs are tiny ---
            for i in range(NPAIR):
                b0, b1 = 2 * i, 2 * i + 1
                h2gf_pass(b0, nsub=2 if b0 <= 2 else 1)
                h2gf_pass(b1, nsub=2 if b1 <= 2 else 1)
                build(b0)
                build(b1)

            # --- phase B: supertile GEMMs; assembly pipelined one pair
            # behind so its c_sb inputs are fully drained from PSUM ---
            c_pass(0, 0)
            c_pass(0, 1)
            for i in range(1, NPAIR):
                c_pass(i, 0)
                c_pass(i, 1)
                asm(2 * (i - 1), i - 1)
                asm(2 * i - 1, i - 1)
            asm(2 * (NPAIR - 1), NPAIR - 1)
            asm(2 * NPAIR - 1, NPAIR - 1)

    nc.compile()
    return nc


def _host_prep(inputs):
    """Prepare per-core input maps.  Returns None if the relation
    structure does not admit the packed-supertile layout (caller falls
    back to the numpy path)."""
    rf = np.asarray(inputs["region_feats"], dtype=np.float32)
    W = np.asarray(inputs["W_conv"], dtype=np.float32)
    Wg = np.asarray(inputs["W_g"], dtype=np.float32)
    blab = np.asarray(inputs["b_lab"], dtype=np.float32)
    bglab = np.asarray(inputs["b_glab"], dtype=np.float32)
    rels = np.asarray(inputs["rels"])
    preds = np.asarray(inputs["pred_classes"])

    rels_r = rels.reshape(N_IMG, RPI, 3)[:, :NUM_REL].reshape(-1, 3)
    preds_r = preds.reshape(N_IMG, RPI)[:, :NUM_REL].reshape(-1)

    # per-pair packing of unique edge sources (k=0: obj, k=1: subj)
    gsrc = np.zeros((NPAIR, 2, 128), np.int64)
    uco_h = np.zeros((EPB, NBLK * 2 * 128), np.float32)
    tgto_h = np.zeros((EPB, NBLK * 2 * 128), np.float32)
    srct_h = np.zeros((128, NBLK * 2 * EPB), np.float32)
    p1h_h = np.zeros((EPB, NBLK * NPRED), np.float32)
    e = np.arange(EPB)
    for i in range(NPAIR):
        for k in range(2):
            off = 0
            for b in (2 * i, 2 * i + 1):
                eb = rels_r[b * EPB:(b + 1) * EPB]
                pb = preds_r[b * EPB:(b + 1) * EPB]
                s = eb[:, 1] - b * 128
                o = eb[:, 2] - b * 128
                src = o if k == 0 else s
                tgt = s if k == 0 else o
                uniq, inv = np.unique(src, return_inverse=True)
                if off + len(uniq) > 128:
                    return None
                gsrc[i, k, off:off + len(uniq)] = uniq + b * 128
                uco_h[e, (b * 2 + k) * 128 + off + inv] = 1.0
                tgto_h[e, (b * 2 + k) * 128 + tgt] = 1.0
                srct_h[src, (b * 2 + k) * EPB + e] = 1.0
                if k == 0:
                    p1h_h[e, b * NPRED + pb] = 1.0
                off += len(uniq)

    # RF^T tiles: rft_h[b, p, d*128+j] = RF[b*128+j, d*128+p]
    rft_h = np.ascontiguousarray(
        rf.T.reshape(32, 128, NBLK, 128).transpose(2, 1, 0, 3), dtype=BF
    ).reshape(NBLK, 128, 32 * 128)

    # gathered supertiles: gat_h[i*2+k, p, d*128+c] = RF[gsrc[i,k,c], d*128+p]
    G = rf[gsrc.reshape(-1)].reshape(NPAIR, 2, 128, 32, 128)  # [i,k,c,d,p]
    gat_h = np.ascontiguousarray(
        G.transpose(0, 1, 4, 3, 2), dtype=BF
    ).reshape(NPAIR * 2, 128, 32 * 128)

    # W slices per core: w_h[p, ((k*32+d)*CW)+j] = W[d*128+p, k*D + c*CW + j]
    Wr = W.reshape(32, 128, 3, NCORES, CW)
    w_cores = [
        np.ascontiguousarray(Wr[:, :, :, c, :].transpose(1, 2, 0, 3),
                             dtype=BF).reshape(128, 3 * 32 * CW)
        for c in range(NCORES)
    ]
    wg_h = np.ascontiguousarray(
        Wg.reshape(32, 128, 3).transpose(1, 0, 2), dtype=BF
    ).reshape(128, 32 * 3)
    blab_cores = [
        np.ascontiguousarray(blab[:, c * CW:(c + 1) * CW], dtype=BF)
        for c in range(NCORES)
    ]
    bgb_h = np.ascontiguousarray(
        np.repeat(bglab.reshape(1, NPRED), 128, axis=0), dtype=BF)
    p1hs_h = np.zeros((128, NPRED), np.float32)
    p1hs_h[:, 0] = 1.0

    shared = {
        "rft": rft_h,
        "gat": gat_h,
        "wg": wg_h,
        "bgb": bgb_h,
        "srct": srct_h.astype(BF),
        "uco": uco_h.astype(BF),
        "tgto": tgto_h.astype(BF),
        "p1h": p1h_h.astype(BF),
        "p1hs": p1hs_h.astype(BF),
        "ident": np.eye(128, dtype=np.float32).astype(BF),
    }
    in_maps = []
    for c in range(NCORES):
        m = dict(shared)
        m["w"] = w_cores[c]
        m["blab"] = blab_cores[c]
        in_maps.append(m)
    return in_maps


def _rels_are_blocked(rels):
    """Check each image's relations reference only that image's regions."""
    rels = np.asarray(rels)
    if rels.shape != (N_IMG * RPI, 3):
        return False
    rels_r = rels.reshape(N_IMG, RPI, 3)[:, :NUM_REL]
    img = np.arange(N_IMG)[:, None]
    lo, hi = img * REG, (img + 1) * REG
    so = rels_r[:, :, 1:3]
    return bool(np.all((so >= lo[:, :, None]) & (so < hi[:, :, None])))


def _numpy_fallback(inputs):
    """Reference-equivalent host computation (only used if the per-image
    relation structure assumption is violated)."""
    rf = np.asarray(inputs["region_feats"], dtype=np.float32)
    W = np.asarray(inputs["W_conv"], dtype=np.float32)
    Wg = np.asarray(inputs["W_g"], dtype=np.float32)
    blab = np.asarray(inputs["b_lab"], dtype=np.float32)
    bglab = np.asarray(inputs["b_glab"], dtype=np.float32)
    rels = np.asarray(inputs["rels"])
    preds = np.asarray(inputs["pred_classes"])
    rels_r = rels.reshape(N_IMG, RPI, 3)[:, :NUM_REL].reshape(-1, 3)
    preds_r = preds.reshape(N_IMG, RPI)[:, :NUM_REL].reshape(-1)
    nf = (rf @ W).reshape(-1, D)
    gfe = (rf @ Wg).reshape(-1)
    s, o = rels_r[:, 1], rels_r[:, 2]
    self_ids = np.arange(N)
    idx = np.concatenate([o * 3 + 0, s * 3 + 1, self_ids * 3 + 2])
    pr = np.concatenate([preds_r, preds_r, np.zeros(N, preds_r.dtype)])
    tgt = np.concatenate([s, o, self_ids])
    gate = 1.0 / (1.0 + np.exp(-(gfe[idx] + bglab[pr, 0])))
    msg = gate[:, None] * (nf[idx] + blab[pr])
    upd = np.zeros((N, D), np.float32)
    np.add.at(upd, tgt, msg)
    return np.maximum(upd, 0.0)


def _run(inputs, trace=False):
    from concourse.bass_utils import run_bass_kernel_spmd

    in_maps = _host_prep(inputs)
    if in_maps is None:
        return None, None
    if "nc" not in _prog_cache:
        _prog_cache["nc"] = _build_program()
    nc = _prog_cache["nc"]
    try:
        res = run_bass_kernel_spmd(nc, in_maps, core_ids=list(range(NCORES)),
                                   trace=trace)
    except Exception:
        # transient device errors (e.g. NRT_EXEC_UNIT_UNRECOVERABLE) have
        # been observed to clear on retry
        import time
        time.sleep(5)
        res = run_bass_kernel_spmd(nc, in_maps, core_ids=list(range(NCORES)),
                                   trace=trace)
    out = np.empty((N, D), np.float32)
    for c in range(NCORES):
        out[:, c * CW:(c + 1) * CW] = (
            np.asarray(res.results[c]["out"]).astype(np.float32)
            .reshape(N, CW))
    return out, res


def kernel(**inputs):
    if not _rels_are_blocked(inputs["rels"]):
        return _numpy_fallback(inputs)
    out, _ = _run(inputs, trace=False)
    if out is None:
        return _numpy_fallback(inputs)
    return out


# revision 46
# speedup vs baseline: 1.0101x; 1.0101x over previous
"""GCN message-passing kernel for Trainium2 (8 NeuronCores, SPMD).

Math (matches the reference):
    gf   = RF @ W_g                          (2048, 3)   gate features
    H_k  = RF @ W_k                          (2048, 4096) per edge type k in {0,1,2}
    gate(e) = sigmoid(gf[src_e, k_e] + b_glab[p_e])
    upd[t]  = sum_{e->t} gate(e) * (H_{k_e}[src_e] + b_lab[p_e])
    out  = relu(upd)

Each image's graph is self-contained (32 regions/image, 20 edges/image),
so with 4 images per 128-row block only the *unique source regions* of a
block's edges (~60, max observed 67) need H_k rows for edge types
k=0,1.  Two blocks' unique sources are packed into one 128-row
"supertile" (8 supertiles per k instead of 16 full blocks), cutting the
dominant RF@W GEMM from 48 to 32 tile passes:
    C_k(pair)  = packed_sources(pair) @ W_k          (128, 512/core)
    out_b      = sum_k T''_k(b) @ C_k + G(b) @ b_lab + diag(g2) H_2(b)
where T''_k (128x128, zero-padded) carries gate values at
(target, packed_source_row) and is built ON DEVICE from gf with one-hot
constant matrices (host only prepares 0/1 index matrices and the packed
gather of RF rows; all data-dependent FLOPs run on Trainium).
The self-loop type k=2 touches every region, so H_2 stays a full
per-block GEMM, interleaved with the gf matmuls that share its lhsT.

Sharding: the output D dim (4096) is split 8 ways -> each core computes
all 2048 rows x its 512 columns, holding a (4096 x 3*512) slice of
W_conv.  No collectives needed; host concatenates the column slices.
"""

import numpy as np
import ml_dtypes

# problem constants (hardcoded per contract)
N_IMG = 64
REG = 32
RPI = 32
NUM_REL = 20
D = 4096
NPRED = 81
N = N_IMG * REG          # 2048
NCORES = 8
CW = D // NCORES         # 512 output cols per core
NBLK = N // 128          # 16 row blocks
NPAIR = NBLK // 2        # 8 supertile pairs
IPB = 128 // REG         # 4 images per block
EPB = IPB * NUM_REL      # 80 edges per block per edge type

BF = ml_dtypes.bfloat16

_prog_cache = {}


def _build_program():
    import concourse.bass as bass
    import concourse.tile as tile
    from concourse import bacc, mybir

    bf16 = mybir.dt.bfloat16
    f32 = mybir.dt.float32
    AF = mybir.ActivationFunctionType
    ALU = mybir.AluOpType

    nc = bacc.Bacc("TRN2", target_bir_lowering=False, debug=False,
                   num_devices=NCORES)

    QW = 8 * 128  # quarter-block tile width (8 d-tiles)
    rft = nc.dram_tensor("rft", [NBLK, 128, 32 * 128], bf16, kind="ExternalInput").ap()
    gat = nc.dram_tensor("gat", [NPAIR * 2, 128, 32 * 128], bf16, kind="ExternalInput").ap()
    w = nc.dram_tensor("w", [128, 3 * 32 * CW], bf16, kind="ExternalInput").ap()
    wg = nc.dram_tensor("wg", [128, 32 * 3], bf16, kind="ExternalInput").ap()
    blab = nc.dram_tensor("blab", [NPRED, CW], bf16, kind="ExternalInput").ap()
    bgb = nc.dram_tensor("bgb", [128, NPRED], bf16, kind="ExternalInput").ap()
    srct = nc.dram_tensor("srct", [128, NBLK * 2 * EPB], bf16, kind="ExternalInput").ap()
    uco = nc.dram_tensor("uco", [EPB, NBLK * 2 * 128], bf16, kind="ExternalInput").ap()
    tgto = nc.dram_tensor("tgto", [EPB, NBLK * 2 * 128], bf16, kind="ExternalInput").ap()
    p1h = nc.dram_tensor("p1h", [EPB, NBLK * NPRED], bf16, kind="ExternalInput").ap()
    p1hs = nc.dram_tensor("p1hs", [128, NPRED], bf16, kind="ExternalInput").ap()
    ident = nc.dram_tensor("ident", [128, 128], bf16, kind="ExternalInput").ap()
    # bf16 output halves the write-drain; host upcasts to f32 (the extra
    # ~0.2% rounding is far inside the accuracy gate)
    out = nc.dram_tensor("out", [NBLK, 128, CW], bf16, kind="ExternalOutput").ap()

    with tile.TileContext(nc) as tc:
        with (
            tc.tile_pool(name="consts", bufs=1) as cpool,
            tc.tile_pool(name="rftq", bufs=12) as rpool,
            tc.tile_pool(name="gatq", bufs=8) as gpool,
            tc.tile_pool(name="csb", bufs=3) as cspool,
            tc.tile_pool(name="blk", bufs=NBLK) as bpool,
            tc.tile_pool(name="bld", bufs=NBLK) as dpool,
            tc.tile_pool(name="small", bufs=2) as spool,
            tc.tile_pool(name="osb", bufs=2) as opool,
            tc.tile_pool(name="pc", bufs=2, space="PSUM") as pcp,
            tc.tile_pool(name="pgf", bufs=1, space="PSUM") as pgfp,
            tc.tile_pool(name="prg", bufs=1, space="PSUM") as prgp,
            tc.tile_pool(name="pgt", bufs=1, space="PSUM") as pgtp,
            tc.tile_pool(name="pmt", bufs=1, space="PSUM") as pmtp,
            tc.tile_pool(name="pout", bufs=2, space="PSUM") as poutp,
        ):
            # --- w chunks: 4 d-tiles each, per k; fine-grained for early start
            WCH = 4 * CW
            w_ch = [[cpool.tile([128, WCH], bf16, tag=f"w{k}c{g}",
                                name=f"w{k}c{g}") for g in range(8)]
                    for k in range(3)]

            def _load_wk(k, eng):
                for g in range(8):
                    eng.dma_start(
                        out=w_ch[k][g][:],
                        in_=w[:, (k * 32 + 4 * g) * CW:(k * 32 + 4 * g + 4) * CW])

            rft_tiles, gat_tiles = {}, {}

            def _load_rft(b, eng=None):
                eng = eng or nc.sync
                qs = []
                for q in range(4):
                    t = rpool.tile([128, QW], bf16, tag="rftq",
                                   name=f"rft{b}_{q}")
                    eng.dma_start(out=t[:],
                                  in_=rft[b, :, q * QW:(q + 1) * QW])
                    qs.append(t)
                rft_tiles[b] = qs

            def _load_gat(i, k):
                qs = []
                for q in range(4):
                    t = gpool.tile([128, QW], bf16, tag="gatq",
                                   name=f"gat{i}_{k}_{q}")
                    nc.sync.dma_start(out=t[:],
                                      in_=gat[i * 2 + k, :, q * QW:(q + 1) * QW])
                    qs.append(t)
                gat_tiles[(i, k)] = qs

            # --- input DMAs on two HW queues:
            #  sync queue:   w2 + the rft/gat bulk streams in consumption
            #                order (pool-ring throttled)
            #  scalar queue: gate-build consts first (cheap, needed by the
            #                builds interleaved into phase A), then W0/W1;
            #                immune to the bulk stream's head-of-line
            #                throttling, arrives during the H2 phase ---
            wg_sb = cpool.tile([128, 32 * 3], bf16, tag="wg")
            nc.scalar.dma_start(out=wg_sb[:], in_=wg[:])
            bgb_sb = cpool.tile([128, NPRED], bf16, tag="bgb")
            nc.scalar.dma_start(out=bgb_sb[:], in_=bgb[:])
            p1hs_sb = cpool.tile([128, NPRED], bf16, tag="p1hs")
            nc.scalar.dma_start(out=p1hs_sb[:], in_=p1hs[:])
            ident_sb = cpool.tile([128, 128], bf16, tag="ident")
            nc.scalar.dma_start(out=ident_sb[:], in_=ident[:])
            blab_sb = cpool.tile([NPRED, CW], bf16, tag="blab")
            nc.scalar.dma_start(out=blab_sb[:], in_=blab[:])
            srct_sb = cpool.tile([128, NBLK * 2 * EPB], bf16, tag="srct")
            nc.scalar.dma_start(out=srct_sb[:], in_=srct[:])
            uco_sb = cpool.tile([EPB, NBLK * 2 * 128], bf16, tag="uco")
            nc.scalar.dma_start(out=uco_sb[:], in_=uco[:])
            tgto_sb = cpool.tile([EPB, NBLK * 2 * 128], bf16, tag="tgto")
            nc.scalar.dma_start(out=tgto_sb[:], in_=tgto[:])
            p1h_sb = cpool.tile([EPB, NBLK * NPRED], bf16, tag="p1h")
            nc.scalar.dma_start(out=p1h_sb[:], in_=p1h[:])
            # interleave rft0 with w2 so block 0's two sub-accumulations
            # gate on ~2.6MB / ~5.2MB instead of all of w2+rft0
            rft0_q = []
            def _load_rft0_q(q):
                t = rpool.tile([128, QW], bf16, tag="rftq", name=f"rft0_{q}")
                nc.sync.dma_start(out=t[:], in_=rft[0, :, q * QW:(q + 1) * QW])
                rft0_q.append(t)
            _load_rft0_q(0)
            _load_rft0_q(1)
            for g in range(4):
                nc.sync.dma_start(
                    out=w_ch[2][g][:],
                    in_=w[:, (2 * 32 + 4 * g) * CW:(2 * 32 + 4 * g + 4) * CW])
            _load_rft0_q(2)
            _load_rft0_q(3)
            for g in range(4, 8):
                nc.sync.dma_start(
                    out=w_ch[2][g][:],
                    in_=w[:, (2 * 32 + 4 * g) * CW:(2 * 32 + 4 * g + 4) * CW])
            rft_tiles[0] = rft0_q
            for b in range(1, NBLK):
                _load_rft(b)
            # W0/W1 ride the sync queue behind the rft stream: the rft ring
            # throttling delays them past phase A's critical window, still
            # well before the first c_pass needs them.
            _load_wk(0, nc.sync)
            _load_wk(1, nc.sync)
            for i in range(NPAIR):
                _load_gat(i, 0)
                _load_gat(i, 1)

            gf_tiles, g2_tiles, h2s_tiles = {}, {}, {}
            c_sb, tT_tiles, gt_tiles = {}, {}, {}

            def rft_lhsT(b, d):
                return rft_tiles[b][d // 8][:, (d % 8) * 128:(d % 8 + 1) * 128]

            def gat_lhsT(i, k, d):
                return gat_tiles[(i, k)][d // 8][:, (d % 8) * 128:(d % 8 + 1) * 128]

            from concourse.tile_rust import add_dep_helper

            # Global PE ordering chain: the static tile scheduler otherwise
            # reorders PE instructions against the DMA arrival plan, creating
            # avoidable stalls.  sync=False deps are compile-time ordering
            # only (no hardware semaphores).
            pe_prev = [None]

            def mm(*args, chain=True, **kw):
                nc.tensor.matmul(*args, **kw)
                inst = nc.main_func.blocks[-1].instructions[-1]
                assert inst.opcode == "Matmult"
                if chain:
                    if pe_prev[0] is not None:
                        add_dep_helper(inst, pe_prev[0], sync=False,
                                       reason="pe-order")
                    pe_prev[0] = inst
                return inst

            def h2gf_pass(b, nsub=1):
                """H_2(b) = RF_b @ W_2 with gf(b) interleaved.  The gf
                matmul reuses the H matmul's stationary operand (same rft
                tile) via ldweights=False; the ordering chain keeps each gf
                adjacent to its H partner so the weights are still loaded
                when it executes.  The self-loop gate is folded into the
                PSUM->SBUF copy: h2s = diag(g2) @ H_2.

                nsub>1 splits the accumulation into sub PSUM groups so the
                pass's DMA gate is per-piece (only used for block 0, where
                the hoisted whole-pass wait would stall startup).  Subs 2/3
                borrow the pout pool's banks, idle during phase A."""
                span = 32 // nsub
                phs = []
                pgf_t = pgfp.tile([128, 12], f32, tag="pgf", name=f"pgf{b}")
                for s in range(nsub):
                    pool, ptag = (pcp, "pc") if s < 2 else (poutp, "pout")
                    ph_t = pool.tile([128, CW], f32, tag=ptag,
                                     name=f"ph{b}_{s}")
                    pgf_ap = pgf_t[:, s * 3:(s + 1) * 3]
                    for dd in range(span):
                        d = s * span + dd
                        lhsT = rft_lhsT(b, d)
                        mm(ph_t[:], lhsT,
                           w_ch[2][d // 4][:, (d % 4) * CW:(d % 4 + 1) * CW],
                           start=(dd == 0), stop=(dd == span - 1))
                        gf_inst = mm(pgf_ap, lhsT,
                                     wg_sb[:, d * 3:(d + 1) * 3],
                                     start=(dd == 0), stop=(dd == span - 1))
                        gf_inst.ldweights = False
                    phs.append(ph_t)
                gf_sb = bpool.tile([128, 3], f32, tag="gf", name=f"gf{b}")
                nc.vector.tensor_copy(out=gf_sb[:], in_=pgf_t[:, 0:3])
                for s in range(1, nsub):
                    nc.vector.tensor_add(gf_sb[:], gf_sb[:],
                                         pgf_t[:, s * 3:(s + 1) * 3])
                gf_tiles[b] = gf_sb
                g2 = bpool.tile([128, 1], f32, tag="g2", name=f"g2_{b}")
                nc.scalar.activation(g2[:], bgb_sb[:, 0:1], AF.Sigmoid,
                                     bias=gf_sb[:, 2:3])
                g2_tiles[b] = g2
                h2s = bpool.tile([128, CW], bf16, tag="h2s", name=f"h2s{b}")
                nc.vector.tensor_scalar_mul(h2s[:], phs[0][:], g2[:])
                for s in range(1, nsub):
                    tmp = spool.tile([128, CW], bf16, tag="h2tmp",
                                     name=f"h2tmp{b}_{s}")
                    nc.vector.tensor_scalar_mul(tmp[:], phs[s][:], g2[:])
                    nc.vector.tensor_add(h2s[:], h2s[:], tmp[:])
                h2s_tiles[b] = h2s
                del rft_tiles[b]

            def c_pass(i, k):
                """C_k(i) = packed_sources(i, k) @ W_k  (supertile GEMM)."""
                pc_t = pcp.tile([128, CW], f32, tag="pc", name=f"pc{i}_{k}")
                for d in range(32):
                    mm(pc_t[:], gat_lhsT(i, k, d),
                       w_ch[k][d // 4][:, (d % 4) * CW:(d % 4 + 1) * CW],
                       start=(d == 0), stop=(d == 31))
                ck = cspool.tile([128, CW], bf16, tag=f"c{k}", name=f"c{i}_{k}")
                nc.vector.tensor_copy(out=ck[:], in_=pc_t[:])
                c_sb[(i, k)] = ck
                del gat_tiles[(i, k)]

            def build(b):
                """Gates -> packed-row scatter matrices T''_k and G^T."""
                gf_sb = gf_tiles[b]
                pgt_t = pgtp.tile([NPRED, 128], f32, tag="pgt", name=f"pgt{b}")
                for k in range(2):
                    sg = spool.tile([128, NPRED], bf16, tag=f"sig{k}",
                                    name=f"sig{b}_{k}")
                    nc.scalar.activation(sg[:], bgb_sb[:], AF.Sigmoid,
                                         bias=gf_sb[:, k:k + 1])
                    prg_t = prgp.tile([EPB, NPRED], f32, tag="prg",
                                      name=f"prg{b}_{k}")
                    mm(prg_t[:],
                       srct_sb[:, (b * 2 + k) * EPB:(b * 2 + k + 1) * EPB],
                       sg[:], start=True, stop=True, chain=False)
                    pg = spool.tile([EPB, NPRED], bf16, tag="pg",
                                    name=f"pg{b}_{k}")
                    nc.vector.tensor_mul(
                        pg[:], prg_t[:],
                        p1h_sb[:, b * NPRED:(b + 1) * NPRED])
                    mm(pgt_t[:], pg[:],
                       tgto_sb[:, (b * 2 + k) * 128:(b * 2 + k + 1) * 128],
                       start=(k == 0), stop=False, chain=False)
                    gcol = spool.tile([EPB, 1], f32, tag="gcol",
                                      name=f"gcol{b}_{k}")
                    nc.vector.tensor_reduce(gcol[:], pg[:],
                                            axis=mybir.AxisListType.X,
                                            op=ALU.add)
                    ucog = spool.tile([EPB, 128], bf16, tag="ucog",
                                      name=f"ucog{b}_{k}")
                    nc.vector.tensor_scalar_mul(
                        ucog[:],
                        uco_sb[:, (b * 2 + k) * 128:(b * 2 + k + 1) * 128],
                        gcol[:])
                    pmt_t = pmtp.tile([128, 128], f32, tag="pmt",
                                      name=f"pmt{b}_{k}")
                    mm(pmt_t[:], ucog[:],
                       tgto_sb[:, (b * 2 + k) * 128:(b * 2 + k + 1) * 128],
                       start=True, stop=True, chain=False)
                    tT = dpool.tile([128, 128], bf16, tag=f"t{k}",
                                    name=f"tT{b}_{k}")
                    nc.vector.tensor_copy(out=tT[:], in_=pmt_t[:])
                    tT_tiles[(b, k)] = tT
                # self-loop: G row 0 += g2
                pg2 = spool.tile([128, NPRED], bf16, tag="pg2", name=f"pg2_{b}")
                nc.vector.tensor_scalar_mul(pg2[:], p1hs_sb[:], g2_tiles[b][:])
                mm(pgt_t[:], pg2[:], ident_sb[:], start=False, stop=True,
                   chain=False)
                gt_sb = dpool.tile([NPRED, 128], bf16, tag="gt", name=f"gt{b}")
                nc.vector.tensor_copy(out=gt_sb[:], in_=pgt_t[:])
                gt_tiles[b] = gt_sb

            def asm(b, i):
                pout_t = poutp.tile([128, CW], f32, tag="pout", name=f"po{b}")
                for k in range(2):
                    mm(pout_t[:], tT_tiles[(b, k)][:], c_sb[(i, k)][:],
                       start=(k == 0), stop=False, chain=False)
                mm(pout_t[:], gt_tiles[b][:], blab_sb[:],
                   start=False, stop=True, chain=False)
                nc.vector.tensor_add(pout_t[:], pout_t[:], h2s_tiles[b][:])
                out_sb = opool.tile([128, CW], bf16, tag="out", name=f"ob{b}")
                nc.scalar.activation(out_sb[:], pout_t[:], AF.Relu)
                nc.scalar.dma_start(out=out[b], in_=out_sb[:])
                del tT_tiles[(b, 0)], tT_tiles[(b, 1)], gt_tiles[b]
                del gf_tiles[b], g2_tiles[b], h2s_tiles[b]
                if b % 2 == 1:
                    del c_sb[(i, 0)], c_sb[(i, 1)]

            # --- phase A: all H2+gf passes (only need w2 + the rft stream;
            # cheapest PE-work-per-DMA-byte, hides the W0/W1/gat/const
            # DMAs).  Gate builds interleave here: they only need gf +
            # consts, and their outputs are tiny ---
            for i in range(NPAIR):
                b0, b1 = 2 * i, 2 * i + 1
                h2gf_pass(b0, nsub=2 if b0 == 0 else 1)
                h2gf_pass(b1)
                build(b0)
                build(b1)

            # --- phase B: supertile GEMMs; assembly pipelined one pair
            # behind so its c_sb inputs are fully drained from PSUM ---
            c_pass(0, 0)
            c_pass(0, 1)
            for i in range(1, NPAIR):
                c_pass(i, 0)
                c_pass(i, 1)
                asm(2 * (i - 1), i - 1)
                asm(2 * i - 1, i - 1)
            asm(2 * (NPAIR - 1), NPAIR - 1)
            asm(2 * NPAIR - 1, NPAIR - 1)

    nc.compile()
    return nc


def _host_prep(inputs):
    """Prepare per-core input maps.  Returns None if the relation
    structure does not admit the packed-supertile layout (caller falls
    back to the numpy path)."""
    rf = np.asarray(inputs["region_feats"], dtype=np.float32)
    W = np.asarray(inputs["W_conv"], dtype=np.float32)
    Wg = np.asarray(inputs["W_g"], dtype=np.float32)
    blab = np.asarray(inputs["b_lab"], dtype=np.float32)
    bglab = np.asarray(inputs["b_glab"], dtype=np.float32)
    rels = np.asarray(inputs["rels"])
    preds = np.asarray(inputs["pred_classes"])

    rels_r = rels.reshape(N_IMG, RPI, 3)[:, :NUM_REL].reshape(-1, 3)
    preds_r = preds.reshape(N_IMG, RPI)[:, :NUM_REL].reshape(-1)

    # per-pair packing of unique edge sources (k=0: obj, k=1: subj)
    gsrc = np.zeros((NPAIR, 2, 128), np.int64)
    uco_h = np.zeros((EPB, NBLK * 2 * 128), np.float32)
    tgto_h = np.zeros((EPB, NBLK * 2 * 128), np.float32)
    srct_h = np.zeros((128, NBLK * 2 * EPB), np.float32)
    p1h_h = np.zeros((EPB, NBLK * NPRED), np.float32)
    e = np.arange(EPB)
    for i in range(NPAIR):
        for k in range(2):
            off = 0
            for b in (2 * i, 2 * i + 1):
                eb = rels_r[b * EPB:(b + 1) * EPB]
                pb = preds_r[b * EPB:(b + 1) * EPB]
                s = eb[:, 1] - b * 128
                o = eb[:, 2] - b * 128
                src = o if k == 0 else s
                tgt = s if k == 0 else o
                uniq, inv = np.unique(src, return_inverse=True)
                if off + len(uniq) > 128:
                    return None
                gsrc[i, k, off:off + len(uniq)] = uniq + b * 128
                uco_h[e, (b * 2 + k) * 128 + off + inv] = 1.0
                tgto_h[e, (b * 2 + k) * 128 + tgt] = 1.0
                srct_h[src, (b * 2 + k) * EPB + e] = 1.0
                if k == 0:
                    p1h_h[e, b * NPRED + pb] = 1.0
                off += len(uniq)

    # RF^T tiles: rft_h[b, p, d*128+j] = RF[b*128+j, d*128+p]
    rft_h = np.ascontiguousarray(
        rf.T.reshape(32, 128, NBLK, 128).transpose(2, 1, 0, 3), dtype=BF
    ).reshape(NBLK, 128, 32 * 128)

    # gathered supertiles: gat_h[i*2+k, p, d*128+c] = RF[gsrc[i,k,c], d*128+p]
    G = rf[gsrc.reshape(-1)].reshape(NPAIR, 2, 128, 32, 128)  # [i,k,c,d,p]
    gat_h = np.ascontiguousarray(
        G.transpose(0, 1, 4, 3, 2), dtype=BF
    ).reshape(NPAIR * 2, 128, 32 * 128)

    # W slices per core: w_h[p, ((k*32+d)*CW)+j] = W[d*128+p, k*D + c*CW + j]
    Wr = W.reshape(32, 128, 3, NCORES, CW)
    w_cores = [
        np.ascontiguousarray(Wr[:, :, :, c, :].transpose(1, 2, 0, 3),
                             dtype=BF).reshape(128, 3 * 32 * CW)
        for c in range(NCORES)
    ]
    wg_h = np.ascontiguousarray(
        Wg.reshape(32, 128, 3).transpose(1, 0, 2), dtype=BF
    ).reshape(128, 32 * 3)
    blab_cores = [
        np.ascontiguousarray(blab[:, c * CW:(c + 1) * CW], dtype=BF)
        for c in range(NCORES)
    ]
    bgb_h = np.ascontiguousarray(
        np.repeat(bglab.reshape(1, NPRED), 128, axis=0), dtype=BF)
    p1hs_h = np.zeros((128, NPRED), np.float32)
    p1hs_h[:, 0] = 1.0

    shared = {
        "rft": rft_h,
        "gat": gat_h,
        "wg": wg_h,
        "bgb": bgb_h,
        "srct": srct_h.astype(BF),
        "uco": uco_h.astype(BF),
        "tgto": tgto_h.astype(BF),
        "p1h": p1h_h.astype(BF),
        "p1hs": p1hs_h.astype(BF),
        "ident": np.eye(128, dtype=np.float32).astype(BF),
    }
    in_maps = []
    for c in range(NCORES):
        m = dict(shared)
        m["w"] = w_cores[c]
        m["blab"] = blab_cores[c]
        in_maps.append(m)
    return in_maps


def _rels_are_blocked(rels):
    """Check each image's relations reference only that image's regions."""
    rels = np.asarray(rels)
    if rels.shape != (N_IMG * RPI, 3):
        return False
    rels_r = rels.reshape(N_IMG, RPI, 3)[:, :NUM_REL]
    img = np.arange(N_IMG)[:, None]
    lo, hi = img * REG, (img + 1) * REG
    so = rels_r[:, :, 1:3]
    return bool(np.all((so >= lo[:, :, None]) & (so < hi[:, :, None])))


def _numpy_fallback(inputs):
    """Reference-equivalent host computation (only used if the per-image
    relation structure assumption is violated)."""
    rf = np.asarray(inputs["region_feats"], dtype=np.float32)
    W = np.asarray(inputs["W_conv"], dtype=np.float32)
    Wg = np.asarray(inputs["W_g"], dtype=np.float32)
    blab = np.asarray(inputs["b_lab"], dtype=np.float32)
    bglab = np.asarray(inputs["b_glab"], dtype=np.float32)
    rels = np.asarray(inputs["rels"])
    preds = np.asarray(inputs["pred_classes"])
    rels_r = rels.reshape(N_IMG, RPI, 3)[:, :NUM_REL].reshape(-1, 3)
    preds_r = preds.reshape(N_IMG, RPI)[:, :NUM_REL].reshape(-1)
    nf = (rf @ W).reshape(-1, D)
    gfe = (rf @ Wg).reshape(-1)
    s, o = rels_r[:, 1], rels_r[:, 2]
    self_ids = np.arange(N)
    idx = np.concatenate([o * 3 + 0, s * 3 + 1, self_ids * 3 + 2])
    pr = np.concatenate([preds_r, preds_r, np.zeros(N, preds_r.dtype)])
    tgt = np.concatenate([s, o, self_ids])
    gate = 1.0 / (1.0 + np.exp(-(gfe[idx] + bglab[pr, 0])))
    msg = gate[:, None] * (nf[idx] + blab[pr])
    upd = np.zeros((N, D), np.float32)
    np.add.at(upd, tgt, msg)
    return np.maximum(upd, 0.0)


def _run(inputs, trace=False):
    from concourse.bass_utils import run_bass_kernel_spmd

    in_maps = _host_prep(inputs)
    if in_maps is None:
        return None, None
    if "nc" not in _prog_cache:
        _prog_cache["nc"] = _build_program()
    nc = _prog_cache["nc"]
    try:
        res = run_bass_kernel_spmd(nc, in_maps, core_ids=list(range(NCORES)),
                                   trace=trace)
    except Exception:
        # transient device errors (e.g. NRT_EXEC_UNIT_UNRECOVERABLE) have
        # been observed to clear on retry
        import time
        time.sleep(5)
        res = run_bass_kernel_spmd(nc, in_maps, core_ids=list(range(NCORES)),
                                   trace=trace)
    out = np.empty((N, D), np.float32)
    for c in range(NCORES):
        out[:, c * CW:(c + 1) * CW] = (
            np.asarray(res.results[c]["out"]).astype(np.float32)
            .reshape(N, CW))
    return out, res


def kernel(**inputs):
    if not _rels_are_blocked(inputs["rels"]):
        return _numpy_fallback(inputs)
    out, _ = _run(inputs, trace=False)
    if out is None:
        return _numpy_fallback(inputs)
    return out
